# revision 2
# baseline (speedup 1.0000x reference)
"""Causal self-attention (B=2, T=2048, C=1024, H=16) on 8 TRN2 NeuronCores.

Sharding: batch x heads — core c owns batch c//4 and heads {4g..4g+3}, g=c%4.
Each core computes a partial o_proj output for its batch [T, C]; the host sums
the 4 partials per batch and adds o_b.

Per-core pipeline (bf16 storage, fp32 PSUM accumulate):
  xT (host-transposed, bf16) --DMA--> SBUF
  q/k/vT = W^T @ x^T              (PE, weights stationary, N=512 moving)
  vaug[h]  = xbar-DMA transpose of vT rows + ones column (denominator trick)
  S^T[j,i] = kT^T qT per j-block  (PE, K=64; causal-trimmed free dim)
  P^T = exp(S^T/8)                (ACT, PSUM->SBUF bf16, [128,1024] groups)
  diagonal blocks masked by a 128x128 triu mask (Pool/DVE)
  O_aug[i, 65] += P^T.T @ vaug    (PE flipped PV: P stationary, vaug moving,
                                   col 64 = softmax denominator for free)
  attO[i, hd] = O * recip(denom)  (DVE per-partition scalar — no broadcast)
  attT = xbar-DMA transpose(attO) (back to [hd, t] for o_proj)
  y[t, e] = attT^T @ ow           (PE, 2 k-steps over 256 head-dims)
"""

import numpy as np

B = 2
T = 2048
C = 1024
H = 16
DH = 64
NCORES = 8
HPC = 4                     # heads per core
CPB = 4                     # cores per batch
TB = T // 128               # 16 t-blocks
KB = C // 128               # 8 contraction blocks for qkv
NCH = T // 512              # 4 i-chunks

_nc_cache = None


def _interleave(primary, filler):
    """Emit primary units with filler units woven in (filler spread evenly)."""
    np_, nf = len(primary), len(filler)
    fi = 0
    for i, u in enumerate(primary):
        u()
        want = int(round((i + 1) * nf / max(np_, 1)))
        while fi < want:
            filler[fi]()
            fi += 1
    while fi < nf:
        filler[fi]()
        fi += 1


def build_bass(dbg=False):
    import concourse.bass as bass
    import concourse.bacc as bacc
    import concourse.tile as tile
    import concourse.mybir as mybir

    F32 = mybir.dt.float32
    BF16 = mybir.dt.bfloat16
    Exp = mybir.ActivationFunctionType.Exp
    Mult = mybir.AluOpType.mult

    nc = bacc.Bacc("TRN2", target_bir_lowering=False, debug=False)

    xT_d = nc.dram_tensor("xT", [C, T], BF16, kind="ExternalInput")
    w_d = nc.dram_tensor("w", [128, KB * 768], BF16, kind="ExternalInput")
    bias_d = nc.dram_tensor("bqkv", [128, 6], F32, kind="ExternalInput")
    ow_d = nc.dram_tensor("ow", [128, 2 * C], BF16, kind="ExternalInput")
    tri_d = nc.dram_tensor("tri", [128, 128], BF16, kind="ExternalInput")
    y_d = nc.dram_tensor("y", [T, C], BF16, kind="ExternalOutput")
    if dbg:
        dbg_d = {nm: nc.dram_tensor(f"dbg_{nm}", [128, 2 * T], BF16,
                                    kind="ExternalOutput")
                 for nm in ("qT", "kT", "vT", "attT")}
        dbg_d["vaug0"] = nc.dram_tensor("dbg_vaug0", [128, TB * 65], BF16,
                                        kind="ExternalOutput")
        dbg_d["attO"] = nc.dram_tensor("dbg_attO", [128, TB * 256], BF16,
                                       kind="ExternalOutput")

    with tile.TileContext(nc) as tc:
        with (
            tc.tile_pool(name="const", bufs=1) as constp,
            tc.tile_pool(name="xT", bufs=1) as xtp,
            tc.tile_pool(name="qkv", bufs=1) as qkvp,
            tc.tile_pool(name="vaug", bufs=1) as vaugp,
            tc.tile_pool(name="vstage", bufs=3) as vstagep,
            tc.tile_pool(name="pT", bufs=4) as ptp,
            tc.tile_pool(name="att", bufs=1) as attp,
            tc.tile_pool(name="recip", bufs=2) as recipp,
            tc.tile_pool(name="yout", bufs=3) as youtp,
            tc.tile_pool(name="ps", bufs=1, space="PSUM") as ps,
        ):
            w_sb = constp.tile([128, KB * 768], BF16)
            ow_sb = constp.tile([128, 2 * C], BF16)
            bias_sb = constp.tile([128, 6], F32)
            tri_sb = constp.tile([128, 128], BF16)
            # w layout: [p, (mt, kb, 128)] — per-m-tile DMA so qkv(0) starts
            # as soon as its own weights land
            w3 = w_sb[:].rearrange("p (mt kb c) -> p mt kb c", kb=KB, c=128)

            def wdma_unit(mt):
                def run():
                    nc.sync.dma_start(
                        w_sb[:, mt * 1024:(mt + 1) * 1024],
                        w_d[:, mt * 1024:(mt + 1) * 1024])
                return run

            def const_dma_unit():
                def run():
                    nc.sync.dma_start(bias_sb[:], bias_d[:])
                    nc.sync.dma_start(tri_sb[:], tri_d[:])
                    nc.sync.dma_start(ow_sb[:], ow_d[:])
                return run

            xT = xtp.tile([128, KB * T], BF16)      # [c, (kb, t)]
            xTv = xT[:].rearrange("p (kb t) -> p kb t", t=T)
            # q/k/vT: [dh (2 heads), (hp, t)] — head h at partition 64*(h%2),
            # free offset (h//2)*T
            qT = qkvp.tile([128, 2 * T], BF16, name="qT")
            kT = qkvp.tile([128, 2 * T], BF16, name="kT")
            vT = qkvp.tile([128, 2 * T], BF16, name="vT")
            # vaug[h]: [t, (tb, 65)] — v natural + ones column (denominator)
            vaug = [vaugp.tile([128, TB * 65], BF16, name=f"vaug{h}")
                    for h in range(HPC)]
            # attO: [t, (tb, 4 heads * 64)] normalized attention out
            attO = attp.tile([128, TB * 256], BF16, name="attO")
            attOv = attO[:].rearrange("p (tb d) -> p tb d", d=256)
            # attT: [hd, (tb, hp, 128)] transposed back for o_proj
            attT = attp.tile([128, 2 * T], BF16, name="attT")

            def xdma_unit(tc_, split=False):
                def run():
                    src = xT_d.rearrange("(kb p) t -> p kb t", p=128)
                    if split:   # per-kb slices: lets qkv start on slice 0
                        for kb in range(KB):
                            nc.sync.dma_start(
                                xTv[:, kb, tc_ * 512:(tc_ + 1) * 512],
                                src[:, kb, tc_ * 512:(tc_ + 1) * 512])
                    else:
                        nc.sync.dma_start(
                            xTv[:, :, tc_ * 512:(tc_ + 1) * 512],
                            src[:, :, tc_ * 512:(tc_ + 1) * 512])
                return run

            def qkv_unit(tc_, mt):
                """One 512-wide t-chunk of one 128-col m-tile (q0,q1,k0,k1,v0,v1)."""
                dstT = (qT, kT, vT)[mt // 2]
                hp = mt % 2

                def run():
                    pt = ps.tile([128, 512], F32, name="psqkv", tag="b1", bufs=2)
                    for kb in range(KB):
                        nc.tensor.matmul(
                            pt[:],
                            w3[:, mt, kb, :],
                            xTv[:, kb, tc_ * 512:(tc_ + 1) * 512],
                            start=(kb == 0),
                            stop=(kb == KB - 1),
                        )
                    nc.vector.tensor_scalar_add(
                        dstT[:, hp * T + tc_ * 512: hp * T + (tc_ + 1) * 512],
                        pt[:], bias_sb[:, mt:mt + 1])
                return run

            def vaug_unit(tc_, h):
                """xbar-transpose v rows for head h, t-chunk tc_, into vaug.

                The XBAR transpose only writes contiguous outputs, so it lands
                in a staging tile; a Pool copy fans it into the 65-stride
                augmented layout."""
                def run():
                    va = vaug[h][:].rearrange("p (tb d) -> p tb d", d=65)
                    if tc_ == 0:
                        nc.vector.memset(va[:, :, 64:65], 1.0)
                    vs = vstagep.tile([128, 4 * 64], BF16, name="vstage",
                                      tag="vs", bufs=3)
                    nc.sync.dma_start_transpose(
                        vs[:].rearrange("p (tb d) -> p tb d", d=64),
                        vT[(h % 2) * 64:(h % 2) * 64 + 64,
                           (h // 2) * T + tc_ * 512:(h // 2) * T + (tc_ + 1) * 512])
                    nc.gpsimd.tensor_copy(
                        va[:, tc_ * 4:(tc_ + 1) * 4, 0:64],
                        vs[:].rearrange("p (tb d) -> p tb d", d=64))
                return run

            def attn_units(ic, heads=range(HPC)):
                """Attention group-units for i-chunk ic (head-sequential)."""
                i0 = 512 * ic
                n_jb = 4 * (ic + 1)
                units = []
                state = {}

                def emit_pv(h, g):
                    """Mask + PV matmuls for group g (exp(g) already issued)."""
                    pv = state[h]
                    pt_sb = state[h, g]
                    for u in range(2):
                        jb = 2 * g + u
                        o = 128 * jb - i0
                        if o >= 0:   # diagonal block: causal mask
                            seg = pt_sb[:, u * 512 + o: u * 512 + o + 128]
                            nc.gpsimd.tensor_tensor(seg, seg, tri_sb[:], Mult)
                        for ib in range(4):
                            if jb > 4 * ic + ib:
                                continue
                            # start=True clears has_written for the WHOLE
                            # bank, so only the first matmul into this pv
                            # tile may use it; the other ib slices start
                            # via overwrite-on-cleared-bits.
                            nc.tensor.matmul(
                                pv[:, ib * 65:(ib + 1) * 65],
                                pt_sb[:, u * 512 + ib * 128:
                                      u * 512 + (ib + 1) * 128],
                                vaug[h][:, jb * 65: jb * 65 + 65],
                                start=(jb == 0 and ib == 0),
                                stop=(jb == 4 * ic + ib),
                                skip_group_check=True,
                            )

                def group(h, g, hq, hf):
                    """S + exp for group g; PV lagged one group behind so the
                    PE never sits in its own FIFO waiting on exp(g)."""
                    if g == 0:
                        state[h] = ps.tile([128, 4 * 65], F32, name="pspv",
                                           tag="pv", bufs=2)
                    sps = ps.tile([128, 1024], F32, name="pssc",
                                  tag="sc", bufs=2)
                    for u in range(2):
                        jb = 2 * g + u
                        o = 128 * jb - i0
                        lo = max(o, 0)
                        nc.tensor.matmul(
                            sps[:, u * 512 + lo:(u + 1) * 512],
                            kT[hq:hq + 64, hf + jb * 128:hf + jb * 128 + 128],
                            qT[hq:hq + 64, hf + i0 + lo:hf + i0 + 512],
                            start=True, stop=True,
                        )
                    pt_sb = ptp.tile([128, 1024], BF16, name="pt",
                                     tag="pt", bufs=6)
                    state[h, g] = pt_sb
                    if 2 * g + 1 == n_jb - 1:
                        # trailing diagonal pair: exp only the causal spans
                        nc.scalar.activation(pt_sb[:, 256:512],
                                             sps[:, 256:512], Exp, scale=0.125)
                        nc.scalar.activation(pt_sb[:, 896:1024],
                                             sps[:, 896:1024], Exp, scale=0.125)
                    else:
                        nc.scalar.activation(pt_sb[:], sps[:], Exp, scale=0.125)
                    if g > 0:
                        emit_pv(h, g - 1)
                    if 2 * g + 1 == n_jb - 1:   # last group: flush + normalize
                        emit_pv(h, g)
                        pv = state[h]
                        rec = recipp.tile([128, 4], F32, name="rec", tag="rc")
                        pvv = pv[:].rearrange("p (ib d) -> p ib d", d=65)
                        nc.vector.reciprocal(
                            rec[:].rearrange("p (ib o) -> p ib o", o=1),
                            pvv[:, :, 64:65])
                        for ib in range(4):
                            tb = ic * 4 + ib
                            nc.vector.tensor_scalar_mul(
                                attOv[:, tb, h * 64:(h + 1) * 64],
                                pvv[:, ib, 0:64],
                                rec[:, ib:ib + 1])

                for h in heads:
                    hq = (h % 2) * 64
                    hf = (h // 2) * T
                    for g in range(n_jb // 2):
                        units.append(lambda h=h, g=g, hq=hq, hf=hf:
                                     group(h, g, hq, hf))
                return units

            def oxpose_unit(ic):
                """One xbar call transposes the whole chunk's attO block."""
                def run():
                    nc.sync.dma_start_transpose(
                        attT[:, ic * 1024:(ic + 1) * 1024]
                        .rearrange("p (g t) -> p g t", t=128),
                        attO[:, ic * 1024:(ic + 1) * 1024])
                return run

            def oproj_unit(tb, act_share=False, split_dma=False):
                def run():
                    yo = youtp.tile([128, C], BF16, name="yo")
                    for ec in range(2):
                        pt = ps.tile([128, 512], F32, name="psy", tag="b1",
                                     bufs=2)
                        for hp in range(2):
                            g = tb * 2 + hp
                            nc.tensor.matmul(
                                pt[:],
                                attT[:, g * 128:(g + 1) * 128],
                                ow_sb[:, hp * C + ec * 512:
                                      hp * C + ec * 512 + 512],
                                start=(hp == 0), stop=(hp == 1),
                            )
                        dst = yo[:, ec * 512:(ec + 1) * 512]
                        if act_share and ec == 1:
                            nc.scalar.activation(
                                dst, pt[:], mybir.ActivationFunctionType.Copy)
                        else:
                            nc.vector.tensor_copy(dst, pt[:])
                        if split_dma:
                            nc.sync.dma_start(
                                y_d[tb * 128:(tb + 1) * 128,
                                    ec * 512:(ec + 1) * 512], dst)
                    if not split_dma:
                        nc.sync.dma_start(
                            y_d[tb * 128:(tb + 1) * 128, :], yo[:])
                return run

            # ---- schedule ----
            # Prologue feeds the hp0 heads of chunk 0 ASAP so ACT starts
            # early; after that each phase interleaves attention(ic) with
            # qkv(ic+1) and oproj(ic-1) so PE stays dense while ACT exps.
            wdma_unit(0)()
            xdma_unit(0, split=True)()
            wdma_unit(2)()
            wdma_unit(4)()
            const_dma_unit()()
            # warm up the PE clock ramp during the initial DMA wait: cheap
            # matmuls on a small memset tile into a scratch psum bank
            warm = constp.tile([128, 8], BF16, name="warm")
            nc.vector.memset(warm[:], 0.0)
            wps = ps.tile([128, 64], F32, name="pswarm", tag="pv", bufs=2)
            for i in range(56):
                nc.tensor.matmul(wps[0:8, 0:8], warm[:, 0:8], warm[:, 0:8],
                                 start=True, stop=True)
            for mt in (0, 2, 4):
                qkv_unit(0, mt)()
            vaug_unit(0, 0)()
            vaug_unit(0, 1)()
            _interleave(attn_units(0, heads=(0, 1)),
                        [wdma_unit(5), qkv_unit(0, 5),
                         vaug_unit(0, 2), vaug_unit(0, 3),
                         wdma_unit(1), qkv_unit(0, 1),
                         wdma_unit(3), qkv_unit(0, 3)])
            _interleave(attn_units(0, heads=(2, 3)),
                        [xdma_unit(1)] +
                        [qkv_unit(1, mt) for mt in (4, 5, 0, 2, 1, 3)] +
                        [vaug_unit(1, h) for h in range(HPC)])
            # phases 1-3: attention(ic) with oproj(ic-2) units spliced in right
            # after each head boundary (the boundary's normalize feeds the DVE
            # queue just ahead of that oproj's evacs — no cross-queue convoy).
            # v-projections go first in the filler so the vaug transposes
            # never block the SP DMA queue.
            for ic in range(1, NCH - 1):
                filler = [xdma_unit(ic + 1), oxpose_unit(ic - 1),
                          qkv_unit(ic + 1, 4), qkv_unit(ic + 1, 5)]
                filler.extend(vaug_unit(ic + 1, h) for h in range(HPC))
                filler.extend(qkv_unit(ic + 1, mt) for mt in (0, 2, 1, 3))
                _interleave(attn_units(ic), filler)
            # phase 3 (ACT-bound): oproj(0..11) woven evenly as PE filler
            oxpose_unit(2)()
            _interleave(attn_units(3),
                        [oproj_unit(tb, act_share=False)
                         for tb in range(12)])
            oxpose_unit(3)()
            for tb in range(12, 16):
                oproj_unit(tb, act_share=True, split_dma=True)()
            if dbg:
                for nm, t in (("qT", qT), ("kT", kT), ("vT", vT),
                              ("attT", attT), ("vaug0", vaug[0]),
                              ("attO", attO)):
                    nc.sync.dma_start(dbg_d[nm][:], t[:])

    nc.compile()
    return nc


def _prep_inputs(x, qkv_w, qkv_b, o_w):
    """Per-core input maps (batch x head sharding), bf16 host-side prep."""
    import ml_dtypes
    bf16 = ml_dtypes.bfloat16

    x = np.asarray(x, dtype=np.float32)
    qkv_w = np.asarray(qkv_w, dtype=np.float32)
    qkv_b = np.asarray(qkv_b, dtype=np.float32)
    o_w = np.asarray(o_w, dtype=np.float32)
    tri = np.triu(np.ones((128, 128), dtype=np.float32)).astype(bf16)

    xT_b = [np.ascontiguousarray(x[b].T.astype(bf16)) for b in range(B)]

    in_maps = []
    for c in range(NCORES):
        b = c // CPB
        lo = (c % CPB) * 256
        # m-tiles: q0,q1,k0,k1,v0,v1 (128 cols each) -> [128, (mt, kb, 128)]
        w_c = np.concatenate(
            [qkv_w[:, lo:lo + 256],
             qkv_w[:, C + lo:C + lo + 256],
             qkv_w[:, 2 * C + lo:2 * C + lo + 256]], axis=1)   # [1024, 768]
        w_c = np.ascontiguousarray(
            w_c.reshape(KB, 128, 6, 128).transpose(1, 2, 0, 3)
            .reshape(128, 6 * KB * 128)).astype(bf16)
        b_c = np.stack(
            [qkv_b[lo:lo + 128], qkv_b[lo + 128:lo + 256],
             qkv_b[C + lo:C + lo + 128], qkv_b[C + lo + 128:C + lo + 256],
             qkv_b[2 * C + lo:2 * C + lo + 128],
             qkv_b[2 * C + lo + 128:2 * C + lo + 256]], axis=1)  # [128, 6]
        ow_c = np.ascontiguousarray(
            o_w[lo:lo + 256, :].reshape(2, 128, C).transpose(1, 0, 2)
            .reshape(128, 2 * C)).astype(bf16)
        in_maps.append({
            "xT": xT_b[b],
            "w": w_c,
            "bqkv": np.ascontiguousarray(b_c, dtype=np.float32),
            "ow": ow_c,
            "tri": tri,
        })
    return in_maps


def kernel(x, qkv_w, qkv_b, o_w, o_b):
    global _nc_cache
    from concourse import bass_utils
    if _nc_cache is None:
        _nc_cache = build_bass()
    nc = _nc_cache
    in_maps = _prep_inputs(x, qkv_w, qkv_b, o_w)
    res = bass_utils.run_bass_kernel_spmd(nc, in_maps, core_ids=list(range(NCORES)))
    o_b = np.asarray(o_b, dtype=np.float64)
    y = np.zeros((B, T, C), dtype=np.float64)
    for c in range(NCORES):
        y[c // CPB] += res.results[c]["y"].astype(np.float64)
    return (y + o_b[None, None, :]).astype(np.float32)


# revision 4
# speedup vs baseline: 1.0137x; 1.0137x over previous
"""Causal self-attention (B=2, T=2048, C=1024, H=16) on 8 TRN2 NeuronCores.

Sharding: batch x heads — core c owns batch c//4 and heads {4g..4g+3}, g=c%4.
Each core computes a partial o_proj output for its batch [T, C]; the host sums
the 4 partials per batch and adds o_b.

Per-core pipeline (bf16 storage, fp32 PSUM accumulate):
  xT (host-transposed, bf16) --DMA--> SBUF
  q/k/vT = W^T @ x^T              (PE, weights stationary, N=512 moving)
  vaug[h]  = xbar-DMA transpose of vT rows + ones column (denominator trick)
  S^T[j,i] = kT^T qT per j-block  (PE, K=64; causal-trimmed free dim)
  P^T = exp(S^T/8)                (ACT, PSUM->SBUF bf16, [128,1024] groups)
  diagonal blocks masked by a 128x128 triu mask (Pool/DVE)
  O_aug[i, 65] += P^T.T @ vaug    (PE flipped PV: P stationary, vaug moving,
                                   col 64 = softmax denominator for free)
  attO[i, hd] = O * recip(denom)  (DVE per-partition scalar — no broadcast)
  attT = xbar-DMA transpose(attO) (back to [hd, t] for o_proj)
  y[t, e] = attT^T @ ow           (PE, 2 k-steps over 256 head-dims)
"""

import numpy as np

B = 2
T = 2048
C = 1024
H = 16
DH = 64
NCORES = 8
HPC = 4                     # heads per core
CPB = 4                     # cores per batch
TB = T // 128               # 16 t-blocks
KB = C // 128               # 8 contraction blocks for qkv
NCH = T // 512              # 4 i-chunks

_nc_cache = None


def _interleave(primary, filler):
    """Emit primary units with filler units woven in (filler spread evenly)."""
    np_, nf = len(primary), len(filler)
    fi = 0
    for i, u in enumerate(primary):
        u()
        want = int(round((i + 1) * nf / max(np_, 1)))
        while fi < want:
            filler[fi]()
            fi += 1
    while fi < nf:
        filler[fi]()
        fi += 1


def build_bass(dbg=False):
    import concourse.bass as bass
    import concourse.bacc as bacc
    import concourse.tile as tile
    import concourse.mybir as mybir

    F32 = mybir.dt.float32
    BF16 = mybir.dt.bfloat16
    FP8 = mybir.dt.float8e4
    DR = mybir.MatmulPerfMode.DoubleRow
    Exp = mybir.ActivationFunctionType.Exp
    Mult = mybir.AluOpType.mult

    nc = bacc.Bacc("TRN2", target_bir_lowering=False, debug=False)

    # x and qkv weights ship as fp8 main + residual (DoubleRow matmuls at
    # half cycles/row; the residual passes recover bf16-level accuracy)
    x8_d = nc.dram_tensor("x8", [C, T], FP8, kind="ExternalInput")
    x8s_d = nc.dram_tensor("x8s", [C, T], FP8, kind="ExternalInput")
    xr8_d = nc.dram_tensor("xr8", [C, T], FP8, kind="ExternalInput")
    ident_d = nc.dram_tensor("ident", [128, 128], BF16, kind="ExternalInput")
    w_d = nc.dram_tensor("w", [128, KB * 768], FP8, kind="ExternalInput")
    r_d = nc.dram_tensor("r", [128, KB * 768], FP8, kind="ExternalInput")
    bias_d = nc.dram_tensor("bqkv", [128, 6], F32, kind="ExternalInput")
    ow_d = nc.dram_tensor("ow", [128, 2 * C], BF16, kind="ExternalInput")
    tri_d = nc.dram_tensor("tri", [128, 128], BF16, kind="ExternalInput")
    y_d = nc.dram_tensor("y", [T, C], BF16, kind="ExternalOutput")
    if dbg:
        dbg_d = {nm: nc.dram_tensor(f"dbg_{nm}", [128, 2 * T], BF16,
                                    kind="ExternalOutput")
                 for nm in ("qT", "kT", "vT", "attT")}
        dbg_d["vaug0"] = nc.dram_tensor("dbg_vaug0", [128, TB * 65], BF16,
                                        kind="ExternalOutput")
        dbg_d["attO"] = nc.dram_tensor("dbg_attO", [128, TB * 256], BF16,
                                       kind="ExternalOutput")

    with tile.TileContext(nc) as tc:
        with (
            tc.tile_pool(name="const", bufs=1) as constp,
            tc.tile_pool(name="xT", bufs=1) as xtp,
            tc.tile_pool(name="qkv", bufs=1) as qkvp,
            tc.tile_pool(name="vaug", bufs=1) as vaugp,
            tc.tile_pool(name="vstage", bufs=3) as vstagep,
            tc.tile_pool(name="pT", bufs=4) as ptp,
            tc.tile_pool(name="att", bufs=1) as attp,
            tc.tile_pool(name="recip", bufs=2) as recipp,
            tc.tile_pool(name="yout", bufs=3) as youtp,
            tc.tile_pool(name="ps", bufs=1, space="PSUM") as ps,
        ):
            w_sb = constp.tile([128, KB * 768], FP8)
            r_sb = constp.tile([128, KB * 768], FP8)
            ow_sb = constp.tile([128, 2 * C], BF16)
            bias_sb = constp.tile([128, 6], F32)
            tri_sb = constp.tile([128, 128], BF16)
            ident_sb = constp.tile([128, 128], BF16)
            # w layout: [p, (mt, kq, pair, 128)] — DoubleRow pairs adjacent
            # kb blocks; per-m-tile DMA so qkv(0) starts as soon as its own
            # weights land
            w3 = w_sb[:].rearrange("p (mt kq pr c) -> p mt kq pr c",
                                   kq=KB // 2, pr=2, c=128)
            r3 = r_sb[:].rearrange("p (mt kq pr c) -> p mt kq pr c",
                                   kq=KB // 2, pr=2, c=128)

            def wdma_unit(mts, res=False):
                def run():
                    sb, d = (r_sb, r_d) if res else (w_sb, w_d)
                    nc.sync.dma_start(
                        sb[:, mts[0] * 1024:mts[1] * 1024],
                        d[:, mts[0] * 1024:mts[1] * 1024])
                return run

            def const_dma_unit():
                def run():
                    nc.sync.dma_start(tri_sb[:], tri_d[:])
                    nc.sync.dma_start(ident_sb[:], ident_d[:])
                    nc.sync.dma_start(ow_sb[:], ow_d[:])
                return run

            xT8 = xtp.tile([128, KB * T], FP8, name="x8")    # [c, (kb, t)]
            xT8s = xtp.tile([128, KB * T], FP8, name="x8s")
            xTr8 = xtp.tile([128, KB * T], FP8, name="xr8")
            xparts = [(xT8, x8_d), (xT8s, x8s_d), (xTr8, xr8_d)]
            xviews = [t[:].rearrange("p (kb t) -> p kb t", t=T)
                      for t, _ in xparts]
            # q/k/vT: [dh (2 heads), (hp, t)] — head h at partition 64*(h%2),
            # free offset (h//2)*T
            qT = qkvp.tile([128, 2 * T], BF16, name="qT")
            kT = qkvp.tile([128, 2 * T], BF16, name="kT")
            vT = qkvp.tile([128, 2 * T], BF16, name="vT")
            # vaug[h]: [t, (tb, 65)] — v natural + ones column (denominator)
            vaug = [vaugp.tile([128, TB * 65], BF16, name=f"vaug{h}")
                    for h in range(HPC)]
            # attO: [t, (tb, 4 heads * 64)] normalized attention out
            attO = attp.tile([128, TB * 256], BF16, name="attO")
            attOv = attO[:].rearrange("p (tb d) -> p tb d", d=256)
            # attT: [hd, (tb, hp, 128)] transposed back for o_proj
            attT = attp.tile([128, 2 * T], BF16, name="attT")

            def xdma_unit(tc_, kbs=None, parts=range(3)):
                def run():
                    for pi in parts:
                        d = xparts[pi][1]
                        dv = xviews[pi]
                        src = d.rearrange("(kb p) t -> p kb t", p=128)
                        if kbs is not None:   # kb sub-range (startup split)
                            nc.sync.dma_start(
                                dv[:, kbs[0]:kbs[1], tc_ * 512:(tc_ + 1) * 512],
                                src[:, kbs[0]:kbs[1], tc_ * 512:(tc_ + 1) * 512])
                        else:
                            nc.sync.dma_start(
                                dv[:, :, tc_ * 512:(tc_ + 1) * 512],
                                src[:, :, tc_ * 512:(tc_ + 1) * 512])
                return run

            def qkv_unit(tc_, mt):
                """One 512-wide t-chunk of one 128-col m-tile (q0,q1,k0,k1,v0,v1)."""
                dstT = (qT, kT, vT)[mt // 2]
                hp = mt % 2

                def run():
                    pt = ps.tile([128, 512], F32, name="psqkv", tag="b1", bufs=2)
                    # 3 DoubleRow passes: x8@W8 + (x8/32)@(32R) + xr8@W8
                    passes = ((w3, xviews[0]), (r3, xviews[1]), (w3, xviews[2]))
                    for pi, (wv, xv) in enumerate(passes):
                        for kq in range(KB // 2):
                            nc.tensor.matmul(
                                pt[:],
                                wv[:, mt, kq],
                                xv[:, 2 * kq:2 * kq + 2,
                                   tc_ * 512:(tc_ + 1) * 512],
                                start=(pi == 0 and kq == 0),
                                stop=(pi == 2 and kq == KB // 2 - 1),
                                perf_mode=DR,
                            )
                    nc.vector.tensor_scalar_add(
                        dstT[:, hp * T + tc_ * 512: hp * T + (tc_ + 1) * 512],
                        pt[:], bias_sb[:, mt:mt + 1])
                return run

            def vaug_unit(tc_, h):
                """xbar-transpose v rows for head h, t-chunk tc_, into vaug.

                The XBAR transpose only writes contiguous outputs, so it lands
                in a staging tile; a Pool copy fans it into the 65-stride
                augmented layout."""
                def run():
                    va = vaug[h][:].rearrange("p (tb d) -> p tb d", d=65)
                    if tc_ == 0:
                        nc.vector.memset(va[:, :, 64:65], 1.0)
                    vs = vstagep.tile([128, 4 * 64], BF16, name="vstage",
                                      tag="vs", bufs=3)
                    nc.sync.dma_start_transpose(
                        vs[:].rearrange("p (tb d) -> p tb d", d=64),
                        vT[(h % 2) * 64:(h % 2) * 64 + 64,
                           (h // 2) * T + tc_ * 512:(h // 2) * T + (tc_ + 1) * 512])
                    nc.gpsimd.tensor_copy(
                        va[:, tc_ * 4:(tc_ + 1) * 4, 0:64],
                        vs[:].rearrange("p (tb d) -> p tb d", d=64))
                return run

            def attn_units(ic, heads=range(HPC)):
                """Attention group-units for i-chunk ic (head-sequential)."""
                i0 = 512 * ic
                n_jb = 4 * (ic + 1)
                units = []
                state = {}

                def emit_pv(h, g):
                    """Mask + PV matmuls for group g (exp(g) already issued)."""
                    pv = state[h]
                    pt_sb = state[h, g]
                    for u in range(2):
                        jb = 2 * g + u
                        o = 128 * jb - i0
                        if o >= 0:   # diagonal block: causal mask
                            seg = pt_sb[:, u * 512 + o: u * 512 + o + 128]
                            nc.vector.tensor_tensor(seg, seg, tri_sb[:], Mult)
                        for ib in range(4):
                            if jb > 4 * ic + ib:
                                continue
                            # start=True clears has_written for the WHOLE
                            # bank, so only the first matmul into this pv
                            # tile may use it; the other ib slices start
                            # via overwrite-on-cleared-bits.
                            nc.tensor.matmul(
                                pv[:, ib * 65:(ib + 1) * 65],
                                pt_sb[:, u * 512 + ib * 128:
                                      u * 512 + (ib + 1) * 128],
                                vaug[h][:, jb * 65: jb * 65 + 65],
                                start=(jb == 0 and ib == 0),
                                stop=(jb == 4 * ic + ib),
                                skip_group_check=True,
                            )

                def group(h, g, hq, hf):
                    """S + exp for group g; PV lagged one group behind so the
                    PE never sits in its own FIFO waiting on exp(g)."""
                    if g == 0:
                        state[h] = ps.tile([128, 4 * 65], F32, name="pspv",
                                           tag="pv", bufs=2)
                    sps = ps.tile([128, 1024], F32, name="pssc",
                                  tag="sc", bufs=2)
                    for u in range(2):
                        jb = 2 * g + u
                        o = 128 * jb - i0
                        lo = max(o, 0)
                        nc.tensor.matmul(
                            sps[:, u * 512 + lo:(u + 1) * 512],
                            kT[hq:hq + 64, hf + jb * 128:hf + jb * 128 + 128],
                            qT[hq:hq + 64, hf + i0 + lo:hf + i0 + 512],
                            start=True, stop=True,
                        )
                    pt_sb = ptp.tile([128, 1024], BF16, name="pt",
                                     tag="pt", bufs=8)
                    state[h, g] = pt_sb
                    if 2 * g + 1 == n_jb - 1:
                        # trailing diagonal pair: exp only the causal spans
                        nc.scalar.activation(pt_sb[:, 256:512],
                                             sps[:, 256:512], Exp, scale=0.125)
                        nc.scalar.activation(pt_sb[:, 896:1024],
                                             sps[:, 896:1024], Exp, scale=0.125)
                    else:
                        nc.scalar.activation(pt_sb[:], sps[:], Exp, scale=0.125)
                    if g > 0:
                        emit_pv(h, g - 1)
                    if 2 * g + 1 == n_jb - 1:   # last group: flush + normalize
                        emit_pv(h, g)
                        pv = state[h]
                        rec = recipp.tile([128, 4], F32, name="rec", tag="rc")
                        pvv = pv[:].rearrange("p (ib d) -> p ib d", d=65)
                        nc.vector.reciprocal(
                            rec[:].rearrange("p (ib o) -> p ib o", o=1),
                            pvv[:, :, 64:65])
                        for ib in range(4):
                            tb = ic * 4 + ib
                            nc.vector.tensor_scalar_mul(
                                attOv[:, tb, h * 64:(h + 1) * 64],
                                pvv[:, ib, 0:64],
                                rec[:, ib:ib + 1])

                for h in heads:
                    hq = (h % 2) * 64
                    hf = (h // 2) * T
                    for g in range(n_jb // 2):
                        units.append(lambda h=h, g=g, hq=hq, hf=hf:
                                     group(h, g, hq, hf))
                return units

            def oxpose_unit(ic):
                """One xbar call transposes the whole chunk's attO block."""
                def run():
                    nc.sync.dma_start_transpose(
                        attT[:, ic * 1024:(ic + 1) * 1024]
                        .rearrange("p (g t) -> p g t", t=128),
                        attO[:, ic * 1024:(ic + 1) * 1024])
                return run

            def oxpose_pe_unit(ic, hp, ibs=range(4)):
                """PE-transpose one head-pair's attO blocks of chunk ic —
                used on the last chunk where the xbar DMA round-trip would
                sit on the critical tail (runs as soon as that head pair's
                normalizes land)."""
                def run():
                    for ib in ibs:
                        tb = ic * 4 + ib
                        pt = ps.tile([128, 128], BF16, name="psxp", tag="b1",
                                     bufs=2)
                        nc.tensor.transpose(
                            pt[:], attOv[:, tb, hp * 128:(hp + 1) * 128],
                            ident_sb[:])
                        nc.vector.tensor_copy(
                            attT[:, (tb * 2 + hp) * 128:(tb * 2 + hp + 1) * 128],
                            pt[:])
                return run

            def oproj_unit(tb, act_share=False, split_dma=False):
                def run():
                    yo = youtp.tile([128, C], BF16, name="yo")
                    for ec in range(2):
                        pt = ps.tile([128, 512], F32, name="psy", tag="b1",
                                     bufs=2)
                        for hp in range(2):
                            g = tb * 2 + hp
                            nc.tensor.matmul(
                                pt[:],
                                attT[:, g * 128:(g + 1) * 128],
                                ow_sb[:, hp * C + ec * 512:
                                      hp * C + ec * 512 + 512],
                                start=(hp == 0), stop=(hp == 1),
                            )
                        dst = yo[:, ec * 512:(ec + 1) * 512]
                        if act_share and ec == 1:
                            nc.scalar.activation(
                                dst, pt[:], mybir.ActivationFunctionType.Copy)
                        else:
                            nc.vector.tensor_copy(dst, pt[:])
                    if split_dma:   # epilogue: issue from the idle ACT queue
                        nc.scalar.dma_start(
                            y_d[tb * 128:(tb + 1) * 128, :], yo[:])
                    else:
                        nc.sync.dma_start(
                            y_d[tb * 128:(tb + 1) * 128, :], yo[:])
                return run

            # ---- schedule ----
            # Prologue feeds the hp0 heads of chunk 0 ASAP so ACT starts
            # early; after that each phase interleaves attention(ic) with
            # qkv(ic+1) and oproj(ic-1) so PE stays dense while ACT exps.
            # startup DMA order: weights/bias land just before each consumer
            # while the x slices stream in between (few large DMAs — the
            # HWDGE issue path serializes at ~625ns per DMA)
            wdma_unit((0, 6))()
            xdma_unit(0, kbs=(0, 4), parts=(0,))()
            wdma_unit((0, 6), res=True)()
            xdma_unit(0, kbs=(4, 8), parts=(0,))()
            nc.sync.dma_start(bias_sb[:], bias_d[:])
            xdma_unit(0, parts=(1, 2))()
            const_dma_unit()()
            # warm up the PE clock ramp during the initial DMA wait: cheap
            # matmuls on a small memset tile into a scratch psum bank
            warm = constp.tile([128, 24], BF16, name="warm")
            nc.vector.memset(warm[:], 0.0)

            def keepalive(n, nn=8):
                wps = ps.tile([128, 64], F32, name="pswarm", tag="pv", bufs=2)
                for i in range(n):
                    nc.tensor.matmul(wps[0:nn, 0:nn], warm[:, 0:nn],
                                     warm[:, 0:nn], start=True, stop=True)

            keepalive(64, 24)
            for mt in (0, 2, 4):
                qkv_unit(0, mt)()
            vaug_unit(0, 0)()
            vaug_unit(0, 1)()
            _interleave(attn_units(0, heads=(0, 1)),
                        [qkv_unit(0, 5),
                         vaug_unit(0, 2), vaug_unit(0, 3),
                         qkv_unit(0, 1), qkv_unit(0, 3)])
            _interleave(attn_units(0, heads=(2, 3)),
                        [xdma_unit(1)] +
                        [qkv_unit(1, mt) for mt in (4, 5, 0, 2, 1, 3)] +
                        [vaug_unit(1, h) for h in range(HPC)])
            # phases 1-3: attention(ic) with oproj(ic-2) units spliced in right
            # after each head boundary (the boundary's normalize feeds the DVE
            # queue just ahead of that oproj's evacs — no cross-queue convoy).
            # v-projections go first in the filler so the vaug transposes
            # never block the SP DMA queue.
            # phase 1
            filler = [xdma_unit(2), oxpose_unit(0),
                      qkv_unit(2, 4), qkv_unit(2, 5)]
            filler.extend(vaug_unit(2, h) for h in range(HPC))
            filler.extend(qkv_unit(2, mt) for mt in (0, 2, 1, 3))
            _interleave(attn_units(1), filler)
            keepalive(8)
            # phase 2
            filler = [xdma_unit(3), oxpose_unit(1),
                      qkv_unit(3, 4), qkv_unit(3, 5)]
            filler.extend(vaug_unit(3, h) for h in range(HPC))
            filler.extend(qkv_unit(3, mt) for mt in (0, 2, 1, 3))
            filler.extend([oproj_unit(0, act_share=False),
                           oproj_unit(1, act_share=False)])
            _interleave(attn_units(2), filler)
            keepalive(8)
            # phase 3 (ACT-bound): oproj(2..11) as PE filler, back-loaded
            oxpose_unit(2)()
            a3 = attn_units(3)
            _interleave(a3[:16], [oproj_unit(tb, act_share=False)
                                  for tb in range(2, 7)])
            a3b = [oxpose_pe_unit(3, 0)] + a3[16:]
            _interleave(a3b, [oproj_unit(tb, act_share=False)
                              for tb in range(7, 12)])
            for tb in range(12, 16):
                oxpose_pe_unit(3, 1, ibs=(tb - 12,))()
                oproj_unit(tb, act_share=True, split_dma=True)()
            if dbg:
                for nm, t in (("qT", qT), ("kT", kT), ("vT", vT),
                              ("attT", attT), ("vaug0", vaug[0]),
                              ("attO", attO)):
                    nc.sync.dma_start(dbg_d[nm][:], t[:])

    nc.compile()
    return nc


def _prep_inputs(x, qkv_w, qkv_b, o_w):
    """Per-core input maps (batch x head sharding), fp8+bf16 host-side prep."""
    import ml_dtypes
    bf16 = ml_dtypes.bfloat16
    fp8 = ml_dtypes.float8_e4m3

    x = np.asarray(x, dtype=np.float32)
    qkv_w = np.asarray(qkv_w, dtype=np.float32)
    qkv_b = np.asarray(qkv_b, dtype=np.float32)
    o_w = np.asarray(o_w, dtype=np.float32)
    tri = np.triu(np.ones((128, 128), dtype=np.float32)).astype(bf16)
    ident = np.eye(128, dtype=np.float32).astype(bf16)

    x8_b, x8s_b, xr8_b = [], [], []
    for b in range(B):
        xT = np.ascontiguousarray(x[b].T)
        x8 = xT.astype(fp8)
        x8f = x8.astype(np.float32)
        x8_b.append(x8)
        x8s_b.append((x8f / 32).astype(fp8))
        xr8_b.append((xT - x8f).astype(fp8))

    in_maps = []
    for c in range(NCORES):
        b = c // CPB
        lo = (c % CPB) * 256
        # m-tiles: q0,q1,k0,k1,v0,v1 (128 cols each) -> [128, (mt, kb, 128)]
        w_c = np.concatenate(
            [qkv_w[:, lo:lo + 256],
             qkv_w[:, C + lo:C + lo + 256],
             qkv_w[:, 2 * C + lo:2 * C + lo + 256]], axis=1)   # [1024, 768]
        w_c = np.ascontiguousarray(
            w_c.reshape(KB, 128, 6, 128).transpose(1, 2, 0, 3)
            .reshape(128, 6 * KB * 128))
        w8_c = w_c.astype(fp8)
        r8_c = ((w_c - w8_c.astype(np.float32)) * 32).astype(fp8)
        b_c = np.stack(
            [qkv_b[lo:lo + 128], qkv_b[lo + 128:lo + 256],
             qkv_b[C + lo:C + lo + 128], qkv_b[C + lo + 128:C + lo + 256],
             qkv_b[2 * C + lo:2 * C + lo + 128],
             qkv_b[2 * C + lo + 128:2 * C + lo + 256]], axis=1)  # [128, 6]
        ow_c = np.ascontiguousarray(
            o_w[lo:lo + 256, :].reshape(2, 128, C).transpose(1, 0, 2)
            .reshape(128, 2 * C)).astype(bf16)
        in_maps.append({
            "x8": x8_b[b],
            "x8s": x8s_b[b],
            "xr8": xr8_b[b],
            "w": w8_c,
            "r": r8_c,
            "bqkv": np.ascontiguousarray(b_c, dtype=np.float32),
            "ow": ow_c,
            "tri": tri,
            "ident": ident,
        })
    return in_maps


def kernel(x, qkv_w, qkv_b, o_w, o_b):
    global _nc_cache
    from concourse import bass_utils
    if _nc_cache is None:
        _nc_cache = build_bass()
    nc = _nc_cache
    in_maps = _prep_inputs(x, qkv_w, qkv_b, o_w)
    res = bass_utils.run_bass_kernel_spmd(nc, in_maps, core_ids=list(range(NCORES)))
    o_b = np.asarray(o_b, dtype=np.float64)
    y = np.zeros((B, T, C), dtype=np.float64)
    for c in range(NCORES):
        y[c // CPB] += res.results[c]["y"].astype(np.float64)
    return (y + o_b[None, None, :]).astype(np.float32)


# revision 5
# speedup vs baseline: 1.0532x; 1.0390x over previous
"""Causal self-attention (B=2, T=2048, C=1024, H=16) on 8 TRN2 NeuronCores.

Sharding: batch x heads — core c owns batch c//4 and heads {4g..4g+3}, g=c%4.
Each core computes a partial o_proj output for its batch [T, C]; the host sums
the 4 partials per batch and adds o_b.

Per-core pipeline (bf16 storage, fp32 PSUM accumulate):
  xT (host-transposed, bf16) --DMA--> SBUF
  q/k/vT = W^T @ x^T              (PE, weights stationary, N=512 moving)
  vaug[h]  = xbar-DMA transpose of vT rows + ones column (denominator trick)
  S^T[j,i] = kT^T qT per j-block  (PE, K=64; causal-trimmed free dim)
  P^T = exp(S^T/8)                (ACT, PSUM->SBUF bf16, [128,1024] groups)
  diagonal blocks masked by a 128x128 triu mask (Pool/DVE)
  O_aug[i, 65] += P^T.T @ vaug    (PE flipped PV: P stationary, vaug moving,
                                   col 64 = softmax denominator for free)
  attO[i, hd] = O * recip(denom)  (DVE per-partition scalar — no broadcast)
  attT = xbar-DMA transpose(attO) (back to [hd, t] for o_proj)
  y[t, e] = attT^T @ ow           (PE, 2 k-steps over 256 head-dims)
"""

import numpy as np

B = 2
T = 2048
C = 1024
H = 16
DH = 64
NCORES = 8
HPC = 4                     # heads per core
CPB = 4                     # cores per batch
TB = T // 128               # 16 t-blocks
KB = C // 128               # 8 contraction blocks for qkv
NCH = T // 512              # 4 i-chunks

_nc_cache = None


def _interleave(primary, filler):
    """Emit primary units with filler units woven in (filler spread evenly)."""
    np_, nf = len(primary), len(filler)
    fi = 0
    for i, u in enumerate(primary):
        u()
        want = int(round((i + 1) * nf / max(np_, 1)))
        while fi < want:
            filler[fi]()
            fi += 1
    while fi < nf:
        filler[fi]()
        fi += 1


def build_bass(dbg=False):
    import concourse.bass as bass
    import concourse.bacc as bacc
    import concourse.tile as tile
    import concourse.mybir as mybir

    F32 = mybir.dt.float32
    BF16 = mybir.dt.bfloat16
    FP8 = mybir.dt.float8e4
    DR = mybir.MatmulPerfMode.DoubleRow
    Exp = mybir.ActivationFunctionType.Exp
    Mult = mybir.AluOpType.mult

    nc = bacc.Bacc("TRN2", target_bir_lowering=False, debug=False)

    # x and qkv weights ship as fp8 main + residual (DoubleRow matmuls at
    # half cycles/row; the residual passes recover bf16-level accuracy)
    x8_d = nc.dram_tensor("x8", [C, T], FP8, kind="ExternalInput")
    x8s_d = nc.dram_tensor("x8s", [C, T], FP8, kind="ExternalInput")
    xr8_d = nc.dram_tensor("xr8", [C, T], FP8, kind="ExternalInput")
    ident_d = nc.dram_tensor("ident", [128, 128], BF16, kind="ExternalInput")
    w_d = nc.dram_tensor("w", [128, KB * 768], FP8, kind="ExternalInput")
    r_d = nc.dram_tensor("r", [128, KB * 768], FP8, kind="ExternalInput")
    bias_d = nc.dram_tensor("bqkv", [128, 6], F32, kind="ExternalInput")
    ow_d = nc.dram_tensor("ow", [128, 2 * C], BF16, kind="ExternalInput")
    tri_d = nc.dram_tensor("tri", [128, 128], BF16, kind="ExternalInput")
    y_d = nc.dram_tensor("y", [T, C], BF16, kind="ExternalOutput")
    if dbg:
        dbg_d = {nm: nc.dram_tensor(f"dbg_{nm}", [128, 2 * T], BF16,
                                    kind="ExternalOutput")
                 for nm in ("qT", "kT", "vT", "attT")}
        dbg_d["vaug0"] = nc.dram_tensor("dbg_vaug0", [128, TB * 65], BF16,
                                        kind="ExternalOutput")
        dbg_d["attO"] = nc.dram_tensor("dbg_attO", [128, TB * 256], BF16,
                                       kind="ExternalOutput")

    with tile.TileContext(nc) as tc:
        with (
            tc.tile_pool(name="const", bufs=1) as constp,
            tc.tile_pool(name="xT", bufs=1) as xtp,
            tc.tile_pool(name="qkv", bufs=1) as qkvp,
            tc.tile_pool(name="vaug", bufs=1) as vaugp,
            tc.tile_pool(name="vstage", bufs=6) as vstagep,
            tc.tile_pool(name="pT", bufs=8) as ptp,
            tc.tile_pool(name="att", bufs=1) as attp,
            tc.tile_pool(name="recip", bufs=4) as recipp,
            tc.tile_pool(name="yout", bufs=6) as youtp,
            tc.tile_pool(name="ps", bufs=1, space="PSUM") as ps,
        ):
            w_sb = constp.tile([128, KB * 768], FP8)
            r_sb = constp.tile([128, KB * 768], FP8)
            ow_sb = constp.tile([128, 2 * C], BF16)
            bias_sb = constp.tile([128, 6], F32)
            tri_sb = constp.tile([128, 128], BF16)
            ident_sb = constp.tile([128, 128], BF16)
            # w layout: [p, (mt, kq, pair, 128)] — DoubleRow pairs adjacent
            # kb blocks; per-m-tile DMA so qkv(0) starts as soon as its own
            # weights land
            w3 = w_sb[:].rearrange("p (mt kq pr c) -> p mt kq pr c",
                                   kq=KB // 2, pr=2, c=128)
            r3 = r_sb[:].rearrange("p (mt kq pr c) -> p mt kq pr c",
                                   kq=KB // 2, pr=2, c=128)

            def wdma_unit(mts, res=False):
                def run():
                    sb, d = (r_sb, r_d) if res else (w_sb, w_d)
                    nc.sync.dma_start(
                        sb[:, mts[0] * 1024:mts[1] * 1024],
                        d[:, mts[0] * 1024:mts[1] * 1024])
                return run

            def const_dma_unit():
                def run():
                    nc.sync.dma_start(tri_sb[:], tri_d[:])
                    nc.sync.dma_start(ident_sb[:], ident_d[:])
                    nc.sync.dma_start(ow_sb[:], ow_d[:])
                return run

            xT8 = xtp.tile([128, KB * T], FP8, name="x8")    # [c, (kb, t)]
            xT8s = xtp.tile([128, KB * T], FP8, name="x8s")
            xTr8 = xtp.tile([128, KB * T], FP8, name="xr8")
            xparts = [(xT8, x8_d), (xT8s, x8s_d), (xTr8, xr8_d)]
            xviews = [t[:].rearrange("p (kb t) -> p kb t", t=T)
                      for t, _ in xparts]
            # q/k/vT: [dh (2 heads), (hp, t)] — head h at partition 64*(h%2),
            # free offset (h//2)*T
            qT = qkvp.tile([128, 2 * T], BF16, name="qT")
            kT = qkvp.tile([128, 2 * T], BF16, name="kT")
            vT = qkvp.tile([128, 2 * T], BF16, name="vT")
            # vaug[h]: [t, (tb, 65)] — v natural + ones column (denominator)
            vaug = [vaugp.tile([128, TB * 65], BF16, name=f"vaug{h}")
                    for h in range(HPC)]
            # attO: [t, (tb, 4 heads * 64)] normalized attention out
            attO = attp.tile([128, TB * 256], BF16, name="attO")
            attOv = attO[:].rearrange("p (tb d) -> p tb d", d=256)
            # attT: [hd, (tb, hp, 128)] transposed back for o_proj
            attT = attp.tile([128, 2 * T], BF16, name="attT")

            def xdma_unit(tc_, kbs=None, parts=range(3)):
                def run():
                    for pi in parts:
                        d = xparts[pi][1]
                        dv = xviews[pi]
                        src = d.rearrange("(kb p) t -> p kb t", p=128)
                        if kbs is not None:   # kb sub-range (startup split)
                            nc.sync.dma_start(
                                dv[:, kbs[0]:kbs[1], tc_ * 512:(tc_ + 1) * 512],
                                src[:, kbs[0]:kbs[1], tc_ * 512:(tc_ + 1) * 512])
                        else:
                            nc.sync.dma_start(
                                dv[:, :, tc_ * 512:(tc_ + 1) * 512],
                                src[:, :, tc_ * 512:(tc_ + 1) * 512])
                return run

            def qkv_unit(tc_, mt):
                """One 512-wide t-chunk of one 128-col m-tile (q0,q1,k0,k1,v0,v1)."""
                dstT = (qT, kT, vT)[mt // 2]
                hp = mt % 2

                def run():
                    pt = ps.tile([128, 512], F32, name="psqkv", tag="b1", bufs=2)
                    # 3 DoubleRow passes: x8@W8 + (x8/32)@(32R) + xr8@W8
                    passes = ((w3, xviews[0]), (r3, xviews[1]), (w3, xviews[2]))
                    for pi, (wv, xv) in enumerate(passes):
                        for kq in range(KB // 2):
                            nc.tensor.matmul(
                                pt[:],
                                wv[:, mt, kq],
                                xv[:, 2 * kq:2 * kq + 2,
                                   tc_ * 512:(tc_ + 1) * 512],
                                start=(pi == 0 and kq == 0),
                                stop=(pi == 2 and kq == KB // 2 - 1),
                                perf_mode=DR,
                            )
                    nc.vector.tensor_scalar_add(
                        dstT[:, hp * T + tc_ * 512: hp * T + (tc_ + 1) * 512],
                        pt[:], bias_sb[:, mt:mt + 1])
                return run

            def vaug_unit(tc_, h):
                """xbar-transpose v rows for head h, t-chunk tc_, into vaug.

                The XBAR transpose only writes contiguous outputs, so it lands
                in a staging tile; a Pool copy fans it into the 65-stride
                augmented layout."""
                def run():
                    va = vaug[h][:].rearrange("p (tb d) -> p tb d", d=65)
                    if tc_ == 0:
                        nc.vector.memset(va[:, :, 64:65], 1.0)
                    vs = vstagep.tile([128, 4 * 64], BF16, name="vstage",
                                      tag="vs", bufs=6)
                    nc.sync.dma_start_transpose(
                        vs[:].rearrange("p (tb d) -> p tb d", d=64),
                        vT[(h % 2) * 64:(h % 2) * 64 + 64,
                           (h // 2) * T + tc_ * 512:(h // 2) * T + (tc_ + 1) * 512])
                    nc.gpsimd.tensor_copy(
                        va[:, tc_ * 4:(tc_ + 1) * 4, 0:64],
                        vs[:].rearrange("p (tb d) -> p tb d", d=64))
                return run

            def attn_units(ic, heads=range(HPC)):
                """Attention group-units for i-chunk ic (head-sequential)."""
                i0 = 512 * ic
                n_jb = 4 * (ic + 1)
                units = []
                state = {}

                def emit_pv(h, g):
                    """Mask + PV matmuls for group g (exp(g) already issued)."""
                    pv = state[h]
                    pt_sb = state[h, g]
                    for u in range(2):
                        jb = 2 * g + u
                        o = 128 * jb - i0
                        if o >= 0:   # diagonal block: causal mask
                            seg = pt_sb[:, u * 512 + o: u * 512 + o + 128]
                            nc.vector.tensor_tensor(seg, seg, tri_sb[:], Mult)
                        for ib in range(4):
                            if jb > 4 * ic + ib:
                                continue
                            # start=True clears has_written for the WHOLE
                            # bank, so only the first matmul into this pv
                            # tile may use it; the other ib slices start
                            # via overwrite-on-cleared-bits.
                            nc.tensor.matmul(
                                pv[:, ib * 65:(ib + 1) * 65],
                                pt_sb[:, u * 512 + ib * 128:
                                      u * 512 + (ib + 1) * 128],
                                vaug[h][:, jb * 65: jb * 65 + 65],
                                start=(jb == 0 and ib == 0),
                                stop=(jb == 4 * ic + ib),
                                skip_group_check=True,
                            )

                def group(h, g, hq, hf):
                    """S + exp for group g; PV lagged one group behind so the
                    PE never sits in its own FIFO waiting on exp(g)."""
                    if g == 0:
                        state[h] = ps.tile([128, 4 * 65], F32, name="pspv",
                                           tag="pv", bufs=2)
                    sps = ps.tile([128, 1024], F32, name="pssc",
                                  tag="sc", bufs=2)
                    for u in range(2):
                        jb = 2 * g + u
                        o = 128 * jb - i0
                        lo = max(o, 0)
                        nc.tensor.matmul(
                            sps[:, u * 512 + lo:(u + 1) * 512],
                            kT[hq:hq + 64, hf + jb * 128:hf + jb * 128 + 128],
                            qT[hq:hq + 64, hf + i0 + lo:hf + i0 + 512],
                            start=True, stop=True,
                        )
                    pt_sb = ptp.tile([128, 1024], BF16, name="pt",
                                     tag="pt", bufs=8)
                    state[h, g] = pt_sb
                    if 2 * g + 1 == n_jb - 1:
                        # trailing diagonal pair: exp only the causal spans
                        nc.scalar.activation(pt_sb[:, 256:512],
                                             sps[:, 256:512], Exp, scale=0.125)
                        nc.scalar.activation(pt_sb[:, 896:1024],
                                             sps[:, 896:1024], Exp, scale=0.125)
                    else:
                        nc.scalar.activation(pt_sb[:], sps[:], Exp, scale=0.125)
                    if g > 0:
                        emit_pv(h, g - 1)
                    if 2 * g + 1 == n_jb - 1:   # last group: flush + normalize
                        emit_pv(h, g)
                        pv = state[h]
                        rec = recipp.tile([128, 4], F32, name="rec", tag="rc")
                        pvv = pv[:].rearrange("p (ib d) -> p ib d", d=65)
                        nc.vector.reciprocal(
                            rec[:].rearrange("p (ib o) -> p ib o", o=1),
                            pvv[:, :, 64:65])
                        for ib in range(4):
                            tb = ic * 4 + ib
                            nc.vector.tensor_scalar_mul(
                                attOv[:, tb, h * 64:(h + 1) * 64],
                                pvv[:, ib, 0:64],
                                rec[:, ib:ib + 1])

                for h in heads:
                    hq = (h % 2) * 64
                    hf = (h // 2) * T
                    for g in range(n_jb // 2):
                        units.append(lambda h=h, g=g, hq=hq, hf=hf:
                                     group(h, g, hq, hf))
                return units

            def oxpose_unit(ic):
                """One xbar call transposes the whole chunk's attO block."""
                def run():
                    nc.sync.dma_start_transpose(
                        attT[:, ic * 1024:(ic + 1) * 1024]
                        .rearrange("p (g t) -> p g t", t=128),
                        attO[:, ic * 1024:(ic + 1) * 1024])
                return run

            def oxpose_pe_unit(ic, hp, ibs=range(4)):
                """PE-transpose one head-pair's attO blocks of chunk ic —
                used on the last chunk where the xbar DMA round-trip would
                sit on the critical tail (runs as soon as that head pair's
                normalizes land)."""
                def run():
                    for ib in ibs:
                        tb = ic * 4 + ib
                        pt = ps.tile([128, 128], BF16, name="psxp", tag="b1",
                                     bufs=2)
                        nc.tensor.transpose(
                            pt[:], attOv[:, tb, hp * 128:(hp + 1) * 128],
                            ident_sb[:])
                        nc.vector.tensor_copy(
                            attT[:, (tb * 2 + hp) * 128:(tb * 2 + hp + 1) * 128],
                            pt[:])
                return run

            def oproj_unit(tb, act_share=False, split_dma=False):
                def run():
                    yo = youtp.tile([128, C], BF16, name="yo")
                    for ec in range(2):
                        pt = ps.tile([128, 512], F32, name="psy", tag="b1",
                                     bufs=2)
                        for hp in range(2):
                            g = tb * 2 + hp
                            nc.tensor.matmul(
                                pt[:],
                                attT[:, g * 128:(g + 1) * 128],
                                ow_sb[:, hp * C + ec * 512:
                                      hp * C + ec * 512 + 512],
                                start=(hp == 0), stop=(hp == 1),
                            )
                        dst = yo[:, ec * 512:(ec + 1) * 512]
                        if act_share and ec == 1:
                            nc.scalar.activation(
                                dst, pt[:], mybir.ActivationFunctionType.Copy)
                        else:
                            nc.vector.tensor_copy(dst, pt[:])
                    if split_dma:   # epilogue: issue from the idle ACT queue
                        nc.scalar.dma_start(
                            y_d[tb * 128:(tb + 1) * 128, :], yo[:])
                    else:
                        nc.sync.dma_start(
                            y_d[tb * 128:(tb + 1) * 128, :], yo[:])
                return run

            # ---- schedule ----
            # Prologue feeds the hp0 heads of chunk 0 ASAP so ACT starts
            # early; after that each phase interleaves attention(ic) with
            # qkv(ic+1) and oproj(ic-1) so PE stays dense while ACT exps.
            # startup DMA order: weights/bias land just before each consumer
            # while the x slices stream in between (few large DMAs — the
            # HWDGE issue path serializes at ~625ns per DMA)
            wdma_unit((0, 6))()
            xdma_unit(0, kbs=(0, 4), parts=(0,))()
            wdma_unit((0, 6), res=True)()
            xdma_unit(0, kbs=(4, 8), parts=(0,))()
            nc.sync.dma_start(bias_sb[:], bias_d[:])
            xdma_unit(0, parts=(1, 2))()
            const_dma_unit()()
            # warm up the PE clock ramp during the initial DMA wait: cheap
            # matmuls on a small memset tile into a scratch psum bank
            warm = constp.tile([128, 24], BF16, name="warm")
            nc.vector.memset(warm[:], 0.0)

            def keepalive(n, nn=8):
                wps = ps.tile([128, 64], F32, name="pswarm", tag="pv", bufs=2)
                for i in range(n):
                    nc.tensor.matmul(wps[0:nn, 0:nn], warm[:, 0:nn],
                                     warm[:, 0:nn], start=True, stop=True)

            keepalive(64, 24)
            for mt in (0, 2, 4):
                qkv_unit(0, mt)()
            vaug_unit(0, 0)()
            vaug_unit(0, 1)()
            _interleave(attn_units(0, heads=(0, 1)),
                        [qkv_unit(0, 5),
                         vaug_unit(0, 2), vaug_unit(0, 3),
                         qkv_unit(0, 1), qkv_unit(0, 3)])
            _interleave(attn_units(0, heads=(2, 3)),
                        [xdma_unit(1)] +
                        [qkv_unit(1, mt) for mt in (4, 5, 0, 2, 1, 3)] +
                        [vaug_unit(1, h) for h in range(HPC)])
            # phases 1-3: attention(ic) with oproj(ic-2) units spliced in right
            # after each head boundary (the boundary's normalize feeds the DVE
            # queue just ahead of that oproj's evacs — no cross-queue convoy).
            # v-projections go first in the filler so the vaug transposes
            # never block the SP DMA queue.
            # phase 1
            filler = [xdma_unit(2), oxpose_unit(0),
                      qkv_unit(2, 4), qkv_unit(2, 5)]
            filler.extend(vaug_unit(2, h) for h in range(HPC))
            filler.extend(qkv_unit(2, mt) for mt in (0, 2, 1, 3))
            _interleave(attn_units(1), filler)
            keepalive(8)
            # phase 2
            filler = [xdma_unit(3), oxpose_unit(1),
                      qkv_unit(3, 4), qkv_unit(3, 5)]
            filler.extend(vaug_unit(3, h) for h in range(HPC))
            filler.extend(qkv_unit(3, mt) for mt in (0, 2, 1, 3))
            filler.extend([oproj_unit(0, act_share=False),
                           oproj_unit(1, act_share=False)])
            _interleave(attn_units(2), filler)
            keepalive(8)
            # phase 3 (ACT-bound): oproj(2..11) as PE filler, back-loaded
            oxpose_unit(2)()
            a3 = attn_units(3)
            _interleave(a3[:16], [oproj_unit(tb, act_share=False)
                                  for tb in range(2, 7)])
            a3b = [oxpose_pe_unit(3, 0)] + a3[16:]
            _interleave(a3b, [oproj_unit(tb, act_share=False)
                              for tb in range(7, 12)])
            for tb in range(12, 16):
                oxpose_pe_unit(3, 1, ibs=(tb - 12,))()
                oproj_unit(tb, act_share=True, split_dma=True)()
            if dbg:
                for nm, t in (("qT", qT), ("kT", kT), ("vT", vT),
                              ("attT", attT), ("vaug0", vaug[0]),
                              ("attO", attO)):
                    nc.sync.dma_start(dbg_d[nm][:], t[:])

    nc.compile()
    return nc


def _prep_inputs(x, qkv_w, qkv_b, o_w):
    """Per-core input maps (batch x head sharding), fp8+bf16 host-side prep."""
    import ml_dtypes
    bf16 = ml_dtypes.bfloat16
    fp8 = ml_dtypes.float8_e4m3

    x = np.asarray(x, dtype=np.float32)
    qkv_w = np.asarray(qkv_w, dtype=np.float32)
    qkv_b = np.asarray(qkv_b, dtype=np.float32)
    o_w = np.asarray(o_w, dtype=np.float32)
    tri = np.triu(np.ones((128, 128), dtype=np.float32)).astype(bf16)
    ident = np.eye(128, dtype=np.float32).astype(bf16)

    x8_b, x8s_b, xr8_b = [], [], []
    for b in range(B):
        xT = np.ascontiguousarray(x[b].T)
        x8 = xT.astype(fp8)
        x8f = x8.astype(np.float32)
        x8_b.append(x8)
        x8s_b.append((x8f / 32).astype(fp8))
        xr8_b.append((xT - x8f).astype(fp8))

    in_maps = []
    for c in range(NCORES):
        b = c // CPB
        lo = (c % CPB) * 256
        # m-tiles: q0,q1,k0,k1,v0,v1 (128 cols each) -> [128, (mt, kb, 128)]
        w_c = np.concatenate(
            [qkv_w[:, lo:lo + 256],
             qkv_w[:, C + lo:C + lo + 256],
             qkv_w[:, 2 * C + lo:2 * C + lo + 256]], axis=1)   # [1024, 768]
        w_c = np.ascontiguousarray(
            w_c.reshape(KB, 128, 6, 128).transpose(1, 2, 0, 3)
            .reshape(128, 6 * KB * 128))
        w8_c = w_c.astype(fp8)
        r8_c = ((w_c - w8_c.astype(np.float32)) * 32).astype(fp8)
        b_c = np.stack(
            [qkv_b[lo:lo + 128], qkv_b[lo + 128:lo + 256],
             qkv_b[C + lo:C + lo + 128], qkv_b[C + lo + 128:C + lo + 256],
             qkv_b[2 * C + lo:2 * C + lo + 128],
             qkv_b[2 * C + lo + 128:2 * C + lo + 256]], axis=1)  # [128, 6]
        ow_c = np.ascontiguousarray(
            o_w[lo:lo + 256, :].reshape(2, 128, C).transpose(1, 0, 2)
            .reshape(128, 2 * C)).astype(bf16)
        in_maps.append({
            "x8": x8_b[b],
            "x8s": x8s_b[b],
            "xr8": xr8_b[b],
            "w": w8_c,
            "r": r8_c,
            "bqkv": np.ascontiguousarray(b_c, dtype=np.float32),
            "ow": ow_c,
            "tri": tri,
            "ident": ident,
        })
    return in_maps


def kernel(x, qkv_w, qkv_b, o_w, o_b):
    global _nc_cache
    from concourse import bass_utils
    if _nc_cache is None:
        _nc_cache = build_bass()
    nc = _nc_cache
    in_maps = _prep_inputs(x, qkv_w, qkv_b, o_w)
    res = bass_utils.run_bass_kernel_spmd(nc, in_maps, core_ids=list(range(NCORES)))
    o_b = np.asarray(o_b, dtype=np.float64)
    y = np.zeros((B, T, C), dtype=np.float64)
    for c in range(NCORES):
        y[c // CPB] += res.results[c]["y"].astype(np.float64)
    return (y + o_b[None, None, :]).astype(np.float32)
